# revision 1
# baseline (speedup 1.0000x reference)
# NetVLAD pooling kernel for Trainium2 (Bass/Tile), 8-core data-parallel over B.
#
# reference:
#   logits = x @ assign_w + assign_b          # (B, T, K)
#   a = softmax(logits, axis=-1)
#   vlad[b,k,d] = sum_t a[b,t,k] * x[b,t,d] - (sum_t a[b,t,k]) * centroids[k,d]
#   out = l2_normalize(vlad, axis=-1).reshape(B, K*D)
#
# Per-core layout (4 batches each):
#   - x loaded natural [t,d]; PE transposes 128x128 tiles -> xT (d on partitions)
#   - GEMM1: logitsT[k=64, t=512] = w[d,k].T @ xT[d,t]  (PSUM accum over 2 d-chunks)
#   - ACT: e = exp(logitsT + b[k])  (bias is per-partition in this layout)
#   - PE: [eT | s] = e_chunk.T @ [I64 | ones]  -> a-layout [t=128, 64] + row sums
#   - a = eT * (1/s)   (DVE reciprocal + ACT scaled copy)
#   - GEMM2: vlad[k=64, 257] += a[t,k].T @ [x | 1][t, 257]  (accumulated per batch)
#   - epilogue: vlad -= a_sum * centroids; L2-normalize over d; DMA out
#
# softmax max-subtraction is skipped: logits ~ N(0, 0.8^2) so exp() is safe in f32,
# and softmax is shift-invariant (matches the reference up to f32 rounding).

import numpy as np

import concourse.bass as bass
import concourse.tile as tile
from concourse import mybir
from concourse.bass_utils import run_bass_kernel_spmd
from concourse.masks import make_identity

B, T, D, K = 32, 4096, 256, 64
NCORES = 8
BPC = B // NCORES          # batches per core
TBLK = 512                 # tokens per pipeline block
NBLK = T // TBLK
NSUB = TBLK // 128         # 128-token subtiles per block
F32 = mybir.dt.float32
F32R = mybir.dt.float32r

_FNS = mybir.ActivationFunctionType


def _split_multi_waits(nc, max_waits=1):
    """The walrus build in this container rejects instructions carrying more
    than one sync wait ("Too many sync wait commands" in setupSyncWait).
    Tile's kernel-tail drain aggregates one wait per live semaphore, so split
    any multi-wait instruction into a chain of single-wait NOPs in front of it.
    """
    for f in nc.m.functions:
        for blk in f.blocks:
            insts = blk.instructions
            if not any(
                i.sync_info and i.sync_info.on_wait and len(i.sync_info.on_wait) > max_waits
                for i in insts
            ):
                continue
            new = []
            for inst in insts:
                si = inst.sync_info
                if si is not None and si.on_wait and len(si.on_wait) > max_waits:
                    waits = list(si.on_wait)
                    for k, w in enumerate(waits[:-max_waits]):
                        nop = mybir.InstNoOp(name=f"{inst.name}-wsplit{k}", ins=[], outs=[])
                        nop.engine = inst.engine
                        nop.sync_info = mybir.SyncInfo(on_wait=[w], on_update=[])
                        new.append(nop)
                    inst.sync_info = mybir.SyncInfo(
                        on_wait=waits[-max_waits:], on_update=list(si.on_update)
                    )
                new.append(inst)
            blk.instructions = new


def build(reps=1, use_f32r=False):
    nc = bass.Bass()
    x_h = nc.declare_dram_parameter("x", [BPC, T, D], F32, isOutput=False)
    w_h = nc.declare_dram_parameter("assign_w", [D, K], F32, isOutput=False)
    b_h = nc.declare_dram_parameter("assign_b", [K, 1], F32, isOutput=False)
    c_h = nc.declare_dram_parameter("centroids", [K, D], F32, isOutput=False)
    o_h = nc.declare_dram_parameter("out", [BPC, K * D], F32, isOutput=True)

    x_ap, w_ap, b_ap, c_ap, o_ap = (h.ap() for h in (x_h, w_h, b_h, c_h, o_h))

    def mm_dt(ap):
        return ap.bitcast(F32R) if use_f32r else ap

    with tile.TileContext(nc) as tc:
        with (
            tc.tile_pool(name="consts", bufs=1) as consts,
            tc.tile_pool(name="xin", bufs=3) as xin,
            tc.tile_pool(name="xts", bufs=2) as xts,
            tc.tile_pool(name="esb", bufs=2) as esb,
            tc.tile_pool(name="asb", bufs=2) as asb,
            tc.tile_pool(name="epi", bufs=2) as epi,
            tc.tile_pool(name="ps_t", bufs=2, space="PSUM") as ps_t,
            tc.tile_pool(name="ps_l", bufs=2, space="PSUM") as ps_l,
            tc.tile_pool(name="ps_a", bufs=2, space="PSUM") as ps_a,
            tc.tile_pool(name="ps_v", bufs=2, space="PSUM") as ps_v,
        ):
            ident = consts.tile([128, 128], F32)
            make_identity(nc, ident)
            eyeones = consts.tile([K, K + 1], F32)
            make_identity(nc, eyeones[:, 0:K])
            nc.gpsimd.memset(eyeones[:, K : K + 1], 1.0)

            w_sb = consts.tile([128, 2, K], F32)
            nc.sync.dma_start(out=w_sb, in_=w_ap.rearrange("(c p) k -> p c k", p=128))
            b_sb = consts.tile([K, 1], F32)
            nc.sync.dma_start(out=b_sb, in_=b_ap)
            c_sb = consts.tile([K, D], F32)
            nc.sync.dma_start(out=c_sb, in_=c_ap)

            for _rep in range(reps):
                for b_i in range(BPC):
                    v_ps = ps_v.tile([K, D + 1], F32)
                    for blk in range(NBLK):
                        x_t = xin.tile([128, NSUB, D + 1], F32)
                        nc.sync.dma_start(
                            out=x_t[:, :, 0:D],
                            in_=x_ap[b_i, blk * TBLK : (blk + 1) * TBLK, :].rearrange(
                                "(n p) d -> p n d", p=128
                            ),
                        )
                        nc.gpsimd.memset(x_t[:, :, D : D + 1], 1.0)

                        xT_sb = xts.tile([128, 2, TBLK], F32)
                        for jd in range(2):
                            xT_ps = ps_t.tile([128, TBLK], F32)
                            for jt in range(NSUB):
                                nc.tensor.transpose(
                                    out=mm_dt(xT_ps[:, jt * 128 : (jt + 1) * 128]),
                                    in_=mm_dt(x_t[:, jt, jd * 128 : (jd + 1) * 128]),
                                    identity=mm_dt(ident),
                                )
                            nc.vector.tensor_copy(out=xT_sb[:, jd, :], in_=xT_ps)

                        l_ps = ps_l.tile([K, TBLK], F32)
                        for jd in range(2):
                            nc.tensor.matmul(
                                out=l_ps,
                                lhsT=mm_dt(w_sb[:, jd, :]),
                                rhs=mm_dt(xT_sb[:, jd, :]),
                                start=(jd == 0),
                                stop=(jd == 1),
                                skip_group_check=True,
                            )

                        e_sb = esb.tile([K, TBLK], F32)
                        nc.scalar.activation(
                            out=e_sb, in_=l_ps, func=_FNS.Exp, bias=b_sb, scale=1.0
                        )

                        a_ps = ps_a.tile([128, NSUB, K + 1], F32)
                        rs = asb.tile([128, NSUB, 1], F32, tag="rs")
                        a_sb = asb.tile([128, NSUB, K], F32, tag="a")
                        for jt in range(NSUB):
                            nc.tensor.matmul(
                                out=a_ps[:, jt, :],
                                lhsT=e_sb[:, jt * 128 : (jt + 1) * 128],
                                rhs=eyeones,
                                start=True,
                                stop=True,
                                skip_group_check=True,
                            )
                            nc.vector.reciprocal(
                                out=rs[:, jt, :], in_=a_ps[:, jt, K : K + 1]
                            )
                            nc.scalar.activation(
                                out=a_sb[:, jt, :],
                                in_=a_ps[:, jt, 0:K],
                                func=_FNS.Copy,
                                scale=rs[:, jt, :],
                            )
                            nc.tensor.matmul(
                                out=v_ps,
                                lhsT=mm_dt(a_sb[:, jt, :]),
                                rhs=mm_dt(x_t[:, jt, :]),
                                start=(blk == 0 and jt == 0),
                                stop=(blk == NBLK - 1 and jt == NSUB - 1),
                                skip_group_check=True,
                            )

                    # epilogue: vlad = v - a_sum * c, then L2-normalize over d
                    tmp = epi.tile([K, D], F32, tag="tmp")
                    nc.vector.tensor_scalar(
                        out=tmp,
                        in0=c_sb,
                        scalar1=v_ps[:, D : D + 1],
                        scalar2=None,
                        op0=mybir.AluOpType.mult,
                    )
                    v_sb = epi.tile([K, D], F32, tag="v")
                    nc.vector.tensor_sub(out=v_sb, in0=v_ps[:, 0:D], in1=tmp)
                    sq = epi.tile([K, D], F32, tag="sq")
                    ssq = epi.tile([K, 1], F32, tag="ssq")
                    nc.scalar.activation(
                        out=sq, in_=v_sb, func=_FNS.Square, accum_out=ssq
                    )
                    nrm = epi.tile([K, 1], F32, tag="nrm")
                    nc.scalar.activation(out=nrm, in_=ssq, func=_FNS.Sqrt)
                    nc.vector.tensor_scalar_max(out=nrm, in0=nrm, scalar1=1e-12)
                    rn = epi.tile([K, 1], F32, tag="rn")
                    nc.vector.reciprocal(out=rn, in_=nrm)
                    o_sb = epi.tile([K, D], F32, tag="o")
                    nc.vector.tensor_scalar_mul(out=o_sb, in0=v_sb, scalar1=rn)
                    nc.sync.dma_start(
                        out=o_ap[b_i].rearrange("(k d) -> k d", d=D), in_=o_sb
                    )

    _split_multi_waits(nc)
    return nc


_nc_cache = {}


def _get_nc(reps=1, use_f32r=False):
    key = (reps, use_f32r)
    if key not in _nc_cache:
        _nc_cache[key] = build(reps=reps, use_f32r=use_f32r)
    return _nc_cache[key]


def _in_maps(x, centroids, assign_w, assign_b):
    x = np.ascontiguousarray(x, dtype=np.float32)
    w = np.ascontiguousarray(assign_w, dtype=np.float32)
    b = np.ascontiguousarray(assign_b, dtype=np.float32).reshape(K, 1)
    c = np.ascontiguousarray(centroids, dtype=np.float32)
    return [
        {
            "x": x[i * BPC : (i + 1) * BPC],
            "assign_w": w,
            "assign_b": b,
            "centroids": c,
        }
        for i in range(NCORES)
    ]


def kernel(x, centroids, assign_w, assign_b):
    nc = _get_nc()
    res = run_bass_kernel_spmd(
        nc, _in_maps(x, centroids, assign_w, assign_b), core_ids=list(range(NCORES))
    )
    return np.concatenate([res.results[i]["out"] for i in range(NCORES)], axis=0)


# revision 16
# speedup vs baseline: 2.5443x; 2.5443x over previous
# NetVLAD pooling kernel for Trainium2 (Bass/Tile), 8-core data-parallel over B.
#
# reference:
#   logits = x @ assign_w + assign_b          # (B, T, K)
#   a = softmax(logits, axis=-1)
#   vlad[b,k,d] = sum_t a[b,t,k] * x[b,t,d] - (sum_t a[b,t,k]) * centroids[k,d]
#   out = l2_normalize(vlad, axis=-1).reshape(B, K*D)
#
# Per-core layout (4 batches each):
#   - x loaded natural [t,d]; PE transposes 128x128 tiles -> xT (d on partitions)
#   - GEMM1: logitsT[k=64, t=512] = w[d,k].T @ xT[d,t]  (PSUM accum over 2 d-chunks)
#   - ACT: e = exp(logitsT + b[k])  (bias is per-partition in this layout)
#   - PE: [eT | s] = e_chunk.T @ [I64 | ones]  -> a-layout [t=128, 64] + row sums
#   - a = eT * (1/s)   (DVE reciprocal + ACT scaled copy)
#   - GEMM2: vlad[k=64, 257] += a[t,k].T @ [x | 1][t, 257]  (accumulated per batch)
#   - epilogue: vlad -= a_sum * centroids; L2-normalize over d; DMA out
#
# softmax max-subtraction is skipped: logits ~ N(0, 0.8^2) so exp() is safe in f32,
# and softmax is shift-invariant (matches the reference up to f32 rounding).

import numpy as np

import concourse.bass as bass
import concourse.tile as tile
from concourse import mybir
from concourse.bass_utils import run_bass_kernel_spmd
from concourse.masks import make_identity

B, T, D, K = 32, 4096, 256, 64
NCORES = 8
BPC = B // NCORES          # batches per core
TBLK = 512                 # tokens per pipeline block
NBLK = T // TBLK
NSUB = TBLK // 128         # 128-token subtiles per block
F32 = mybir.dt.float32
F32R = mybir.dt.float32r

_FNS = mybir.ActivationFunctionType


def _split_multi_waits(nc, max_waits=1):
    """The walrus build in this container rejects instructions carrying more
    than one sync wait ("Too many sync wait commands" in setupSyncWait).
    Tile's kernel-tail drain aggregates one wait per live semaphore, so split
    any multi-wait instruction into a chain of single-wait NOPs in front of it.
    """
    for f in nc.m.functions:
        for blk in f.blocks:
            insts = blk.instructions
            if not any(
                i.sync_info and i.sync_info.on_wait and len(i.sync_info.on_wait) > max_waits
                for i in insts
            ):
                continue
            new = []
            for inst in insts:
                si = inst.sync_info
                if si is not None and si.on_wait and len(si.on_wait) > max_waits:
                    waits = list(si.on_wait)
                    for k, w in enumerate(waits[:-max_waits]):
                        nop = mybir.InstNoOp(name=f"{inst.name}-wsplit{k}", ins=[], outs=[])
                        nop.engine = inst.engine
                        nop.sync_info = mybir.SyncInfo(on_wait=[w], on_update=[])
                        new.append(nop)
                    inst.sync_info = mybir.SyncInfo(
                        on_wait=waits[-max_waits:], on_update=list(si.on_update)
                    )
                new.append(inst)
            blk.instructions = new


def build(reps=1, use_f32r=False):
    # use_f32r: False -> all f32; True / "gv" -> f32r on GEMM1+GEMM2 only;
    # "all" -> f32r also on the PE transposes.
    f32r_gv = use_f32r in (True, "gv", "all")
    f32r_tr = use_f32r == "all"

    nc = bass.Bass()
    x_h = nc.declare_dram_parameter("x", [BPC, T, D], F32, isOutput=False)
    w_h = nc.declare_dram_parameter("assign_w", [D, K], F32, isOutput=False)
    b_h = nc.declare_dram_parameter("assign_b", [K, 1], F32, isOutput=False)
    c_h = nc.declare_dram_parameter("centroids", [K, D], F32, isOutput=False)
    o_h = nc.declare_dram_parameter("out", [BPC, K * D], F32, isOutput=True)

    x_ap, w_ap, b_ap, c_ap, o_ap = (h.ap() for h in (x_h, w_h, b_h, c_h, o_h))

    def gv_dt(ap):
        return ap.bitcast(F32R) if f32r_gv else ap

    def tr_dt(ap):
        return ap.bitcast(F32R) if f32r_tr else ap

    with tile.TileContext(nc) as tc:
        with (
            tc.tile_pool(name="consts", bufs=1) as consts,
            tc.tile_pool(name="xin", bufs=4) as xin,
            tc.tile_pool(name="xts", bufs=3) as xts,
            tc.tile_pool(name="esb", bufs=3) as esb,
            tc.tile_pool(name="asb", bufs=3) as asb,
            tc.tile_pool(name="epi", bufs=2) as epi,
            tc.tile_pool(name="ps_t", bufs=2, space="PSUM") as ps_t,
            tc.tile_pool(name="ps_l", bufs=2, space="PSUM") as ps_l,
            tc.tile_pool(name="ps_a", bufs=2, space="PSUM") as ps_a,
            tc.tile_pool(name="ps_v", bufs=2, space="PSUM") as ps_v,
        ):
            ident = consts.tile([128, 128], F32)
            make_identity(nc, ident)
            eyeones = consts.tile([K, K + 1], F32)
            make_identity(nc, eyeones[:, 0:K])
            nc.gpsimd.memset(eyeones[:, K : K + 1], 1.0)

            # fp32r matmuls require a full 128-column stationary operand
            # (col_grp == 0xf), so w and a are zero-padded K=64 -> 128.
            w_sb = consts.tile([128, 2, 128], F32)
            nc.sync.dma_start(
                out=gv_dt(w_sb[:, :, 0:K]),
                in_=gv_dt(w_ap.rearrange("(c p) k -> p c k", p=128)),
            )
            ones2 = consts.tile([128, 2], F32)
            nc.gpsimd.memset(ones2, 1.0)
            zpadw = consts.tile([128, 2, K], F32, tag="zpadw")
            nc.gpsimd.memset(zpadw, 0.0)
            nc.vector.tensor_copy(out=gv_dt(w_sb[:, :, K:128]), in_=zpadw)
            zpada = consts.tile([128, NSUB, K], F32, tag="zpada")
            nc.gpsimd.memset(zpada, 0.0)

            b_sb = consts.tile([K, 1], F32)
            nc.sync.dma_start(out=b_sb, in_=b_ap)
            c_sb = consts.tile([K, D], F32)
            nc.sync.dma_start(out=c_sb, in_=c_ap)

            for _rep in range(reps):
                for b_i in range(BPC):
                    v_ps = ps_v.tile([128, D + 2], F32)
                    for blk in range(NBLK):
                        # fp32r moving operand needs an even element count, so
                        # the GEMM2 rhs is [x | 1 | 1] of width 258 (both extra
                        # columns produce a_sum; the last is ignored).
                        x_t = xin.tile([128, NSUB, D + 2], F32)
                        nc.sync.dma_start(
                            out=gv_dt(x_t[:, :, 0:D]),
                            in_=gv_dt(
                                x_ap[b_i, blk * TBLK : (blk + 1) * TBLK, :].rearrange(
                                    "(n p) d -> p n d", p=128
                                )
                            ),
                        )
                        for jt in range(NSUB):
                            nc.vector.tensor_copy(
                                out=gv_dt(x_t[:, jt, D : D + 2]), in_=ones2
                            )

                        xT_sb = xts.tile([128, 2, TBLK], F32)
                        for jd in range(2):
                            xT_ps = ps_t.tile([128, TBLK], F32)
                            for jt in range(NSUB):
                                nc.tensor.transpose(
                                    out=tr_dt(xT_ps[:, jt * 128 : (jt + 1) * 128]),
                                    in_=tr_dt(x_t[:, jt, jd * 128 : (jd + 1) * 128]),
                                    identity=tr_dt(ident),
                                )
                            # split the PSUM->SBUF copies across DVE and ACT;
                            # in f32r mode the copy also rounds to f32r
                            if jd == 0:
                                nc.vector.tensor_copy(
                                    out=gv_dt(xT_sb[:, jd, :]), in_=xT_ps
                                )
                            else:
                                nc.scalar.copy(out=gv_dt(xT_sb[:, jd, :]), in_=xT_ps)

                        l_ps = ps_l.tile([128, TBLK], F32)
                        for jd in range(2):
                            nc.tensor.matmul(
                                out=l_ps,
                                lhsT=gv_dt(w_sb[:, jd, :]),
                                rhs=gv_dt(xT_sb[:, jd, :]),
                                start=(jd == 0),
                                stop=(jd == 1),
                                skip_group_check=True,
                            )

                        e_sb = esb.tile([K, TBLK], F32)
                        nc.scalar.activation(
                            out=e_sb,
                            in_=l_ps[0:K, :],
                            func=_FNS.Exp,
                            bias=b_sb,
                            scale=1.0,
                        )

                        a_ps = ps_a.tile([128, NSUB, K + 1], F32)
                        rs = asb.tile([128, NSUB, 1], F32, tag="rs")
                        a_sb = asb.tile([128, NSUB, 128], F32, tag="a")
                        nc.vector.tensor_copy(
                            out=gv_dt(a_sb[:, :, K:128]), in_=zpada
                        )
                        for jt in range(NSUB):
                            nc.tensor.matmul(
                                out=a_ps[:, jt, :],
                                lhsT=e_sb[:, jt * 128 : (jt + 1) * 128],
                                rhs=eyeones,
                                start=True,
                                stop=True,
                                skip_group_check=True,
                            )
                            nc.vector.reciprocal(
                                out=rs[:, jt, :], in_=a_ps[:, jt, K : K + 1]
                            )
                            nc.vector.tensor_scalar_mul(
                                out=gv_dt(a_sb[:, jt, 0:K]),
                                in0=a_ps[:, jt, 0:K],
                                scalar1=rs[:, jt, :],
                            )
                            nc.tensor.matmul(
                                out=v_ps,
                                lhsT=gv_dt(a_sb[:, jt, :]),
                                rhs=gv_dt(x_t[:, jt, :]),
                                start=(blk == 0 and jt == 0),
                                stop=(blk == NBLK - 1 and jt == NSUB - 1),
                                skip_group_check=True,
                            )

                    # epilogue: vlad = v - a_sum * c, then L2-normalize over d
                    tmp = epi.tile([K, D], F32, tag="tmp")
                    nc.vector.tensor_scalar(
                        out=tmp,
                        in0=c_sb,
                        scalar1=v_ps[0:K, D : D + 1],
                        scalar2=None,
                        op0=mybir.AluOpType.mult,
                    )
                    v_sb = epi.tile([K, D], F32, tag="v")
                    nc.vector.tensor_sub(out=v_sb, in0=v_ps[0:K, 0:D], in1=tmp)
                    sq = epi.tile([K, D], F32, tag="sq")
                    ssq = epi.tile([K, 1], F32, tag="ssq")
                    nc.scalar.activation(
                        out=sq, in_=v_sb, func=_FNS.Square, accum_out=ssq
                    )
                    nrm = epi.tile([K, 1], F32, tag="nrm")
                    nc.scalar.activation(out=nrm, in_=ssq, func=_FNS.Sqrt)
                    nc.vector.tensor_scalar_max(out=nrm, in0=nrm, scalar1=1e-12)
                    rn = epi.tile([K, 1], F32, tag="rn")
                    nc.vector.reciprocal(out=rn, in_=nrm)
                    o_sb = epi.tile([K, D], F32, tag="o")
                    nc.vector.tensor_scalar_mul(out=o_sb, in0=v_sb, scalar1=rn)
                    nc.sync.dma_start(
                        out=o_ap[b_i].rearrange("(k d) -> k d", d=D), in_=o_sb
                    )

    _split_multi_waits(nc)
    return nc


_nc_cache = {}


def _get_nc(reps=1, use_f32r=False):
    key = (reps, use_f32r)
    if key not in _nc_cache:
        _nc_cache[key] = build(reps=reps, use_f32r=use_f32r)
    return _nc_cache[key]


def _in_maps(x, centroids, assign_w, assign_b):
    x = np.ascontiguousarray(x, dtype=np.float32)
    w = np.ascontiguousarray(assign_w, dtype=np.float32)
    b = np.ascontiguousarray(assign_b, dtype=np.float32).reshape(K, 1)
    c = np.ascontiguousarray(centroids, dtype=np.float32)
    return [
        {
            "x": x[i * BPC : (i + 1) * BPC],
            "assign_w": w,
            "assign_b": b,
            "centroids": c,
        }
        for i in range(NCORES)
    ]


def kernel(x, centroids, assign_w, assign_b):
    nc = _get_nc()
    res = run_bass_kernel_spmd(
        nc, _in_maps(x, centroids, assign_w, assign_b), core_ids=list(range(NCORES))
    )
    return np.concatenate([res.results[i]["out"] for i in range(NCORES)], axis=0)
